# revision 11
# baseline (speedup 1.0000x reference)
"""Contrastive-loss-over-memory-bank kernel for 8 Trainium2 NeuronCores.

Math (reference): after scattering the batch into the bank (U[ind]=u,
Y[ind]=y), with dist[i,j] = ||u_i - U_j||^2, t[i,j] = y_i . Y_j (one-hot
labels -> t in {0,1}, and s = 1-t), margin M = 96:

    loss1 = mean_ij[ t*0.5*dist + (1-t)*0.5*relu(M-dist) ]
    loss2 = 0.1 * mean|1-|u||
    out   = loss1 + loss2

Device strategy: shard the bank rows (num_train axis) across 8 cores.
Each core computes, for its 12544 padded bank rows (98 tiles of 128):

    mm1 : dist^T tile [128j, 256i] via one K=50 matmul with augmented
          operands (rows = [U^T; ||U||^2; 1] x [-2u^T; 1; ||u||^2])
    ACT : r = relu(M - dist) (+ free-axis row sums -> Sum_ij r)
    DVE : copy dist^T from PSUM to SBUF
    mm2 : V += Y_aug^T @ [dist^T | r^T]  accumulated in one PSUM bank,
          where Y_aug = [Y | 1]  (101 columns)

V collapses every t-weighted sum: Sum_ij t*dist = sum(y^T * V[:100,:256]),
Sum_ij t*r = sum(y^T * V[:100,256:]). The host sums the 8 per-core V/racc
outputs (the scalar "all-reduce") and combines with loss2.

Bank rows are padded 100000 -> 100352 with U=0, ||U||^2=1e6, Y=0 rows,
which contribute exactly zero to every accumulated quantity.
"""

import os
import sys
from contextlib import ExitStack

import numpy as np

try:
    import concourse.bass as bass
except ImportError:  # set up the in-container toolchain path
    for _p in ("/opt/trn_rl_repo", "/root/.axon_site/_ro/trn_rl_repo"):
        if os.path.isdir(_p) and _p not in sys.path:
            sys.path.insert(0, _p)
    import concourse.bass as bass

import concourse.mybir as mybir
import concourse.tile as tile
from concourse import bacc, bass_utils

# Problem constants (hardcoded per contract)
NUM_TRAIN = 100000
N_CLASS = 100
BIT = 48
BATCH = 256
ALPHA = 0.1
MARGIN = 2.0 * BIT  # 96
N_CORES = 8
JT = 128  # bank rows per tile (partition dim)
TILES_PER_CORE = 98
N_PER_CORE = JT * TILES_PER_CORE  # 12544
N_PAD = N_PER_CORE * N_CORES  # 100352
KAUG = BIT + 2  # 50: [U^T; b; 1]
MAUG = N_CLASS + 1  # 101: [Y | 1]
PAD_B = 1.0e6  # ||U||^2 stand-in for padding rows
CHUNK = 14  # j-tiles per DMA chunk (98 = 7*14)

_state = {"nc": None, "trace": False, "last_result": None}


def _build_program():
    nc = bacc.Bacc("TRN2", target_bir_lowering=False, debug=False)
    f32 = mybir.dt.float32
    ut = nc.dram_tensor("ut", [KAUG, N_PER_CORE], f32, kind="ExternalInput").ap()
    ya = nc.dram_tensor(
        "ya", [JT, TILES_PER_CORE * MAUG], f32, kind="ExternalInput"
    ).ap()
    rhs = nc.dram_tensor("rhs", [KAUG, BATCH], f32, kind="ExternalInput").ap()
    v_out = nc.dram_tensor("v", [MAUG, 2 * BATCH], f32, kind="ExternalOutput").ap()
    racc_out = nc.dram_tensor(
        "racc", [JT, TILES_PER_CORE], f32, kind="ExternalOutput"
    ).ap()

    with tile.TileContext(nc) as tc, ExitStack() as ctx:
        const_pool = ctx.enter_context(tc.tile_pool(name="const", bufs=1))
        in_pool = ctx.enter_context(tc.tile_pool(name="ins", bufs=2))
        dd_pool = ctx.enter_context(tc.tile_pool(name="dd", bufs=4))
        rr_pool = ctx.enter_context(tc.tile_pool(name="rr", bufs=4))
        pd_pool = ctx.enter_context(tc.tile_pool(name="pd", bufs=4, space="PSUM"))
        pv_pool = ctx.enter_context(tc.tile_pool(name="pv", bufs=1, space="PSUM"))
        acc_pool = ctx.enter_context(tc.tile_pool(name="acc", bufs=1))

        rhs_sb = const_pool.tile([KAUG, BATCH], f32)
        nc.sync.dma_start(rhs_sb[:], rhs[:, :])
        racc_sb = acc_pool.tile([JT, TILES_PER_CORE], f32)
        # separate single-writer/single-reader PSUM banks for the two V halves
        psum_va = pv_pool.tile([MAUG, BATCH], f32, tag="va")
        psum_vb = pv_pool.tile([MAUG, BATCH], f32, tag="vb")
        n_chunks = TILES_PER_CORE // CHUNK
        for ch in range(n_chunks):
            ut_sb = in_pool.tile([KAUG, CHUNK * JT], f32, tag="ut")
            nc.sync.dma_start(
                ut_sb[:], ut[:, ch * CHUNK * JT : (ch + 1) * CHUNK * JT]
            )
            ya_sb = in_pool.tile([JT, CHUNK * MAUG], f32, tag="ya")
            nc.sync.dma_start(
                ya_sb[:], ya[:, ch * CHUNK * MAUG : (ch + 1) * CHUNK * MAUG]
            )
            for t in range(CHUNK):
                jt = ch * CHUNK + t
                psum_d = pd_pool.tile([JT, BATCH], f32)
                nc.tensor.matmul(
                    psum_d[:],
                    ut_sb[:, t * JT : (t + 1) * JT],
                    rhs_sb[:],
                    start=True,
                    stop=True,
                )
                ya_t = ya_sb[:, t * MAUG : (t + 1) * MAUG]
                # psum_d = dist - M; d-column adds M back (DVE)
                dd = dd_pool.tile([JT, BATCH], f32)
                nc.vector.tensor_scalar_add(dd[:], psum_d[:], float(MARGIN))
                # relu(M - dist) with free-axis row sums (ACT)
                rr = rr_pool.tile([JT, BATCH], f32)
                nc.scalar.activation(
                    rr[:],
                    psum_d[:],
                    mybir.ActivationFunctionType.Relu,
                    bias=0.0,
                    scale=-1.0,
                    accum_out=racc_sb[:, jt : jt + 1],
                )
                nc.tensor.matmul(
                    psum_va[:],
                    ya_t,
                    dd[:],
                    start=(jt == 0),
                    stop=(jt == TILES_PER_CORE - 1),
                    skip_group_check=True,
                )
                nc.tensor.matmul(
                    psum_vb[:],
                    ya_t,
                    rr[:],
                    start=(jt == 0),
                    stop=(jt == TILES_PER_CORE - 1),
                    skip_group_check=True,
                )
        v_sb = acc_pool.tile([MAUG, 2 * BATCH], f32)
        nc.vector.tensor_copy(v_sb[:, 0:BATCH], psum_va[:])
        nc.scalar.copy(v_sb[:, BATCH : 2 * BATCH], psum_vb[:])
        # separate DMAs: one producer sem each (walrus wait-slot limits)
        nc.sync.dma_start(v_out[:, 0:BATCH], v_sb[:, 0:BATCH])
        nc.sync.dma_start(v_out[:, BATCH : 2 * BATCH], v_sb[:, BATCH : 2 * BATCH])
        nc.sync.dma_start(racc_out[:, :], racc_sb[:])
    nc.compile()  # bacc: wait legalization (evsems), regalloc, nop fusion
    return nc


def _get_program():
    if _state["nc"] is None:
        _state["nc"] = _build_program()
    return _state["nc"]


def _prepare_in_maps(u, y, ind, U, Y):
    u = np.asarray(u, dtype=np.float32)
    y = np.asarray(y, dtype=np.float32)
    ind = np.asarray(ind)
    U = np.array(U, dtype=np.float32)  # private copies: we scatter in place
    Y = np.array(Y, dtype=np.float32)
    U[ind] = u
    Y[ind] = y
    b = (U * U).sum(axis=1)
    a = (u * u).sum(axis=1)

    rhs_aug = np.empty((KAUG, BATCH), np.float32)
    rhs_aug[:BIT] = (-2.0 * u).T
    rhs_aug[BIT] = 1.0
    rhs_aug[BIT + 1] = a - MARGIN  # psum holds dist - M

    in_maps = []
    for c in range(N_CORES):
        lo, hi = c * N_PER_CORE, (c + 1) * N_PER_CORE
        ncopy = max(0, min(hi, NUM_TRAIN) - lo)  # real rows in this shard
        ut_c = np.zeros((KAUG, N_PER_CORE), np.float32)
        ut_c[BIT] = PAD_B
        ut_c[BIT + 1] = 1.0
        ya_c = np.zeros((JT, TILES_PER_CORE * MAUG), np.float32)
        if ncopy > 0:
            ut_c[:BIT, :ncopy] = U[lo : lo + ncopy].T
            ut_c[BIT, :ncopy] = b[lo : lo + ncopy]
            # SBUF-layout swizzle for Y_aug: [tiles*128, 101] -> [128, tiles*101]
            ya_full = np.zeros((N_PER_CORE, MAUG), np.float32)
            ya_full[:ncopy, :N_CLASS] = Y[lo : lo + ncopy]
            ya_full[:, N_CLASS] = 1.0
            ya_c = np.ascontiguousarray(
                ya_full.reshape(TILES_PER_CORE, JT, MAUG)
                .transpose(1, 0, 2)
                .reshape(JT, TILES_PER_CORE * MAUG)
            )
        in_maps.append({"ut": ut_c, "ya": ya_c, "rhs": rhs_aug})
    return in_maps, u, y


def _combine(results, u, y):
    V = np.zeros((MAUG, 2 * BATCH), np.float64)
    r_sum = 0.0
    for c in range(N_CORES):
        V += results[c]["v"].astype(np.float64)
        r_sum += results[c]["racc"].astype(np.float64).sum()
    yT = y.astype(np.float64).T  # [100, 256]
    sum_td = (yT * V[:N_CLASS, 0:BATCH]).sum()
    sum_tr = (yT * V[:N_CLASS, BATCH : 2 * BATCH]).sum()
    loss1 = 0.5 * (r_sum + sum_td - sum_tr) / (BATCH * NUM_TRAIN)
    loss2 = ALPHA * np.abs(1.0 - np.abs(u.astype(np.float64))).mean()
    return np.array(loss1 + loss2, dtype=np.float32)


def kernel(u, y, ind, U, Y):
    in_maps, u32, y32 = _prepare_in_maps(u, y, ind, U, Y)
    nc = _get_program()
    res = bass_utils.run_bass_kernel_spmd(
        nc, in_maps, core_ids=list(range(N_CORES)), trace=_state["trace"]
    )
    _state["last_result"] = res
    return _combine(res.results, u32, y32)


# revision 12
# speedup vs baseline: 1.4729x; 1.4729x over previous
"""Contrastive-loss-over-memory-bank kernel for 8 Trainium2 NeuronCores.

Math (reference): after scattering the batch into the bank (U[ind]=u,
Y[ind]=y), with dist[i,j] = ||u_i - U_j||^2, t[i,j] = y_i . Y_j (one-hot
labels -> t in {0,1}, and s = 1-t), margin M = 96:

    loss1 = mean_ij[ t*0.5*dist + (1-t)*0.5*relu(M-dist) ]
    loss2 = 0.1 * mean|1-|u||
    out   = loss1 + loss2

Device strategy: shard the bank rows (num_train axis) across 8 cores.
Each core computes, for its 12544 padded bank rows (98 tiles of 128):

    mm1 : dist^T tile [128j, 256i] via one K=50 matmul with augmented
          operands (rows = [U^T; ||U||^2; 1] x [-2u^T; 1; ||u||^2])
    ACT : r = relu(M - dist) (+ free-axis row sums -> Sum_ij r)
    DVE : copy dist^T from PSUM to SBUF
    mm2 : V += Y_aug^T @ [dist^T | r^T]  accumulated in one PSUM bank,
          where Y_aug = [Y | 1]  (101 columns)

V collapses every t-weighted sum: Sum_ij t*dist = sum(y^T * V[:100,:256]),
Sum_ij t*r = sum(y^T * V[:100,256:]). The host sums the 8 per-core V/racc
outputs (the scalar "all-reduce") and combines with loss2.

Bank rows are padded 100000 -> 100352 with U=0, ||U||^2=1e6, Y=0 rows,
which contribute exactly zero to every accumulated quantity.
"""

import os
import sys
from contextlib import ExitStack

import numpy as np
import ml_dtypes

try:
    import concourse.bass as bass
except ImportError:  # set up the in-container toolchain path
    for _p in ("/opt/trn_rl_repo", "/root/.axon_site/_ro/trn_rl_repo"):
        if os.path.isdir(_p) and _p not in sys.path:
            sys.path.insert(0, _p)
    import concourse.bass as bass

import concourse.mybir as mybir
import concourse.tile as tile
from concourse import bacc, bass_utils

# Problem constants (hardcoded per contract)
NUM_TRAIN = 100000
N_CLASS = 100
BIT = 48
BATCH = 256
ALPHA = 0.1
MARGIN = 2.0 * BIT  # 96
N_CORES = 8
JT = 128  # bank rows per tile (partition dim)
TILES_PER_CORE = 98
N_PER_CORE = JT * TILES_PER_CORE  # 12544
N_PAD = N_PER_CORE * N_CORES  # 100352
KAUG = BIT + 2  # 50: [U^T; b; 1]
MAUG = N_CLASS + 1  # 101: [Y | 1]
PAD_B = 1.0e6  # ||U||^2 stand-in for padding rows
CHUNK = 14  # j-tiles per DMA chunk (98 = 7*14)

_state = {"nc": None, "trace": False, "last_result": None}


def _build_program():
    nc = bacc.Bacc("TRN2", target_bir_lowering=False, debug=False)
    f32 = mybir.dt.float32
    bf16 = mybir.dt.bfloat16
    ut = nc.dram_tensor("ut", [KAUG, N_PER_CORE], bf16, kind="ExternalInput").ap()
    ya = nc.dram_tensor(
        "ya", [JT, TILES_PER_CORE * MAUG], bf16, kind="ExternalInput"
    ).ap()
    rhs = nc.dram_tensor("rhs", [KAUG, BATCH], bf16, kind="ExternalInput").ap()
    v_out = nc.dram_tensor("v", [MAUG, 2 * BATCH], f32, kind="ExternalOutput").ap()
    racc_out = nc.dram_tensor(
        "racc", [JT, TILES_PER_CORE], f32, kind="ExternalOutput"
    ).ap()

    with tile.TileContext(nc) as tc, ExitStack() as ctx:
        const_pool = ctx.enter_context(tc.tile_pool(name="const", bufs=1))
        in_pool = ctx.enter_context(tc.tile_pool(name="ins", bufs=2))
        dd_pool = ctx.enter_context(tc.tile_pool(name="dd", bufs=4))
        pd_pool = ctx.enter_context(tc.tile_pool(name="pd", bufs=4, space="PSUM"))
        pv_pool = ctx.enter_context(tc.tile_pool(name="pv", bufs=1, space="PSUM"))
        acc_pool = ctx.enter_context(tc.tile_pool(name="acc", bufs=1))

        rhs_sb = const_pool.tile([KAUG, BATCH], bf16)
        nc.sync.dma_start(rhs_sb[:], rhs[:, :])
        racc_sb = acc_pool.tile([JT, TILES_PER_CORE], f32)
        psum_v = pv_pool.tile([MAUG, 2 * BATCH], f32)
        n_chunks = TILES_PER_CORE // CHUNK
        for ch in range(n_chunks):
            ut_sb = in_pool.tile([KAUG, CHUNK * JT], bf16, tag="ut")
            nc.sync.dma_start(
                ut_sb[:], ut[:, ch * CHUNK * JT : (ch + 1) * CHUNK * JT]
            )
            ya_sb = in_pool.tile([JT, CHUNK * MAUG], bf16, tag="ya")
            nc.sync.dma_start(
                ya_sb[:], ya[:, ch * CHUNK * MAUG : (ch + 1) * CHUNK * MAUG]
            )
            for t in range(CHUNK):
                jt = ch * CHUNK + t
                psum_d = pd_pool.tile([JT, BATCH], f32)
                nc.tensor.matmul(
                    psum_d[:],
                    ut_sb[:, t * JT : (t + 1) * JT],
                    rhs_sb[:],
                    start=True,
                    stop=True,
                )
                ya_t = ya_sb[:, t * MAUG : (t + 1) * MAUG]
                # combined [d | r] tile, bf16, two single-range producers
                dr = dd_pool.tile([JT, 2 * BATCH], bf16)
                # psum_d = dist - M; d-half adds M back (DVE)
                nc.vector.tensor_scalar_add(dr[:, 0:BATCH], psum_d[:], float(MARGIN))
                # r-half: relu(M - dist) with free-axis row sums (ACT)
                nc.scalar.activation(
                    dr[:, BATCH : 2 * BATCH],
                    psum_d[:],
                    mybir.ActivationFunctionType.Relu,
                    bias=0.0,
                    scale=-1.0,
                    accum_out=racc_sb[:, jt : jt + 1],
                )
                nc.tensor.matmul(
                    psum_v[:],
                    ya_t,
                    dr[:],
                    start=(jt == 0),
                    stop=(jt == TILES_PER_CORE - 1),
                    skip_group_check=True,
                )
        v_sb = acc_pool.tile([MAUG, 2 * BATCH], f32)
        nc.vector.tensor_copy(v_sb[:], psum_v[:])
        nc.sync.dma_start(v_out[:, :], v_sb[:])
        nc.sync.dma_start(racc_out[:, :], racc_sb[:])
    nc.compile()  # bacc: wait legalization (evsems), regalloc, nop fusion
    return nc


def _get_program():
    if _state["nc"] is None:
        _state["nc"] = _build_program()
    return _state["nc"]


def _prepare_in_maps(u, y, ind, U, Y):
    u = np.asarray(u, dtype=np.float32)
    y = np.asarray(y, dtype=np.float32)
    ind = np.asarray(ind)
    U = np.array(U, dtype=np.float32)  # private copies: we scatter in place
    Y = np.array(Y, dtype=np.float32)
    U[ind] = u
    Y[ind] = y
    b = (U * U).sum(axis=1)
    a = (u * u).sum(axis=1)

    rhs_aug = np.empty((KAUG, BATCH), np.float32)
    rhs_aug[:BIT] = (-2.0 * u).T
    rhs_aug[BIT] = 1.0
    rhs_aug[BIT + 1] = a - MARGIN  # psum holds dist - M
    rhs_bf = rhs_aug.astype(ml_dtypes.bfloat16)

    in_maps = []
    for c in range(N_CORES):
        lo, hi = c * N_PER_CORE, (c + 1) * N_PER_CORE
        ncopy = max(0, min(hi, NUM_TRAIN) - lo)  # real rows in this shard
        ut_c = np.zeros((KAUG, N_PER_CORE), np.float32)
        ut_c[BIT] = PAD_B
        ut_c[BIT + 1] = 1.0
        ya_c = np.zeros((JT, TILES_PER_CORE * MAUG), np.float32)
        if ncopy > 0:
            ut_c[:BIT, :ncopy] = U[lo : lo + ncopy].T
            ut_c[BIT, :ncopy] = b[lo : lo + ncopy]
            # SBUF-layout swizzle for Y_aug: [tiles*128, 101] -> [128, tiles*101]
            ya_full = np.zeros((N_PER_CORE, MAUG), np.float32)
            ya_full[:ncopy, :N_CLASS] = Y[lo : lo + ncopy]
            ya_full[:, N_CLASS] = 1.0
            ya_c = np.ascontiguousarray(
                ya_full.reshape(TILES_PER_CORE, JT, MAUG)
                .transpose(1, 0, 2)
                .reshape(JT, TILES_PER_CORE * MAUG)
            )
        in_maps.append(
            {
                "ut": ut_c.astype(ml_dtypes.bfloat16),
                "ya": ya_c.astype(ml_dtypes.bfloat16),
                "rhs": rhs_bf,
            }
        )
    return in_maps, u, y


def _combine(results, u, y):
    V = np.zeros((MAUG, 2 * BATCH), np.float64)
    r_sum = 0.0
    for c in range(N_CORES):
        V += results[c]["v"].astype(np.float64)
        r_sum += results[c]["racc"].astype(np.float64).sum()
    yT = y.astype(np.float64).T  # [100, 256]
    sum_td = (yT * V[:N_CLASS, 0:BATCH]).sum()
    sum_tr = (yT * V[:N_CLASS, BATCH : 2 * BATCH]).sum()
    loss1 = 0.5 * (r_sum + sum_td - sum_tr) / (BATCH * NUM_TRAIN)
    loss2 = ALPHA * np.abs(1.0 - np.abs(u.astype(np.float64))).mean()
    return np.array(loss1 + loss2, dtype=np.float32)


def kernel(u, y, ind, U, Y):
    in_maps, u32, y32 = _prepare_in_maps(u, y, ind, U, Y)
    nc = _get_program()
    res = bass_utils.run_bass_kernel_spmd(
        nc, in_maps, core_ids=list(range(N_CORES)), trace=_state["trace"]
    )
    _state["last_result"] = res
    return _combine(res.results, u32, y32)
